# revision 5
# baseline (speedup 1.0000x reference)
"""Multi-head self-attention Trainium2 Bass kernel (8-core SPMD).

Sharding: data-parallel over query rows. The flattened (B*S, D) = (8192, 512)
query space is split into 8 blocks of 1024 rows; core c handles batch c//4,
query rows (c%4)*1024 .. +1024. Each core recomputes K/V for its whole batch
(4-way duplicated) which avoids any cross-core communication; host-side
gather is a pure concatenation.

Layout strategy: activations live transposed in SBUF ([D, S], d on
partitions). Projections then need no weight transposes:
  K^T = Wk^T x^T   (lhsT = Wk chunk, rhs = x^T chunk)
  V   = x Wv       (lhsT = x^T chunk, rhs = Wv chunk)
Scores are computed transposed ([k, q], k on partitions) so softmax's
denominator comes from a ones-column appended to V (row 64 of the attention
output accumulator), and A^T is directly consumable by the A@V matmul.
exp() runs on the scalar engine with the 1/sqrt(dk) folded into its scale.
The normalized per-head outputs O^T are exactly the lhsT the output
projection wants, so no transposes are needed anywhere except on the input x.

Matmul operands are stored as float32r (the PE's full-rate fp32 streaming
format, ~13 mantissa bits): every producer is an engine op whose output
dtype is f32r, which satisfies the BIR verifier's rounding requirement.
DMA-loaded operands (weights/biases) pass through a converting copy.
"""

from contextlib import ExitStack

import numpy as np

import concourse.bass as bass
import concourse.tile as tile
from concourse import bacc, mybir
from concourse.bass_utils import run_bass_kernel_spmd

N_CORES = 8
B, S, D, H, DK = 2, 4096, 512, 8, 64
QL = B * S // N_CORES          # 1024 query rows per core
P = 128
NT_S = S // P                  # 32 sequence tiles
NT_D = D // P                  # 4 d-model chunks
QC = QL // 512                 # 2 query chunks of 512
F32 = mybir.dt.float32
F32R = mybir.dt.float32r
EXP = mybir.ActivationFunctionType.Exp

USE_F32R = True
DTM = F32R if USE_F32R else F32   # dtype for matmul operands


def _emit(ctx: ExitStack, tc: tile.TileContext, io: dict):
    nc = tc.nc
    xb, xq = io["xb"], io["xq"]
    wq, wk, wv, wo = io["wq"], io["wk"], io["wv"], io["wo"]
    bq, bk, bv, bo = io["bq"], io["bk"], io["bv"], io["bo"]
    ident = io["ident"]
    out = io["out"]

    mm = nc.tensor.matmul

    # ---- pools persistent across the whole kernel ------------------------
    consts = ctx.enter_context(tc.tile_pool(name="consts", bufs=1))
    xt_pool = ctx.enter_context(tc.tile_pool(name="xt", bufs=1))
    qt_pool = ctx.enter_context(tc.tile_pool(name="qt", bufs=4))
    # PSUM: proj/transpose/bias-bcast/outproj pool (1 bank x2), scores
    # (2 banks x2), attention accumulators (1 bank x2) -> exactly 8 banks.
    pp = ctx.enter_context(tc.tile_pool(name="pp", bufs=2, space="PSUM"))
    sp_pool = ctx.enter_context(tc.tile_pool(name="sp", bufs=2, space="PSUM"))
    o_pool = ctx.enter_context(tc.tile_pool(name="o", bufs=2, space="PSUM"))

    # ---- constants --------------------------------------------------------
    ident_sb = consts.tile([P, P], F32, tag="ident")
    nc.sync.dma_start(out=ident_sb[:], in_=ident[:])
    ones_f32 = consts.tile([P, 1], F32, tag="ones_f32")
    nc.vector.memset(ones_f32[:], 1.0)
    ones_sb = consts.tile([1, 512], DTM, tag="ones")
    nc.vector.tensor_copy(out=ones_sb[:], in_=ones_f32[0:1, 0:1].broadcast_to([1, 512]))
    # a f32 ones row living on partition 64 (denominator broadcast lhsT)
    ones64_sb = consts.tile([65, 64], F32, tag="ones64")
    nc.vector.memset(ones64_sb[64:65, :], 1.0)
    bias_sb = {}
    with tc.tile_pool(name="stg0", bufs=2) as stg0:
        for nm, ap in (("bq", bq), ("bk", bk), ("bv", bv), ("bo", bo)):
            st = stg0.tile([1, D], F32, tag="bstg")
            nc.sync.dma_start(out=st[:], in_=ap[:])
            t = consts.tile([1, D], DTM, tag=nm)
            nc.vector.tensor_copy(out=t[:], in_=st[:])
            bias_sb[nm] = t

    xT = xt_pool.tile([P, NT_D * S], DTM, tag="xT")
    wq_r = wq.rearrange("(dc p) m -> p dc m", p=P)
    wk_r = wk.rearrange("(dc p) m -> p dc m", p=P)
    wv_r = wv.rearrange("(dc p) m -> p dc m", p=P)
    QT = []

    with tc.tile_pool(name="xq", bufs=1) as xq_pool:
        xqT = xq_pool.tile([P, NT_D * QL], DTM, tag="xqT")

        # ---- stage A: x^T and xq^T via PE transposes ---------------------
        with tc.tile_pool(name="xn", bufs=3) as xn_pool:
            for src_ap, ntile, dst in ((xb, NT_S, xT), (xq, QL // P, xqT)):
                for st in range(ntile):
                    xn = xn_pool.tile([P, D], F32, tag="xn")
                    nc.sync.dma_start(out=xn[:], in_=src_ap[st * P:(st + 1) * P, :])
                    tp = pp.tile([P, 512], F32, tag="proj")
                    for dc in range(NT_D):
                        nc.tensor.transpose(
                            tp[:, dc * P:(dc + 1) * P],
                            xn[:, dc * P:(dc + 1) * P],
                            ident_sb[:],
                        )
                    dst_ap = dst[:, :].rearrange("p (dc s) -> p dc s", dc=NT_D)
                    nc.vector.tensor_copy(
                        out=dst_ap[:, :, st * P:(st + 1) * P],
                        in_=tp[:, :].rearrange("p (dc j) -> p dc j", dc=NT_D),
                    )

        # ---- stage B: Q^T for all 4 head pairs ---------------------------
        with (
            tc.tile_pool(name="wqp", bufs=1) as wq_pool,
            tc.tile_pool(name="stgb", bufs=2) as stgb,
        ):
            for pr in range(4):
                wst = stgb.tile([P, NT_D * P], F32, tag="wstg")
                nc.sync.dma_start(
                    out=wst[:, :].rearrange("p (dc m) -> p dc m", dc=NT_D),
                    in_=wq_r[:, :, pr * P:(pr + 1) * P],
                )
                wqp = wq_pool.tile([P, NT_D * P], DTM, tag="wq")
                nc.vector.tensor_copy(out=wqp[:], in_=wst[:])
                qt = qt_pool.tile([P, QL], DTM, tag="QT")
                for qc in range(QC):
                    ps = pp.tile([P, 512], F32, tag="proj")
                    for dc in range(NT_D):
                        mm(ps[:], wqp[:, dc * P:(dc + 1) * P],
                           xqT[:, dc * QL + qc * 512:dc * QL + (qc + 1) * 512],
                           start=(dc == 0), stop=False)
                    mm(ps[:], bias_sb["bq"][0:1, pr * P:(pr + 1) * P],
                       ones_sb[0:1, :], start=False, stop=True)
                    nc.vector.tensor_copy(out=qt[:, qc * 512:(qc + 1) * 512],
                                          in_=ps[:])
                QT.append(qt)

    # ---- stage C: per 4-head group: V, then per pair K^T + attention -----
    OT = []  # per-head [64, QL] normalized attention outputs (transposed)
    with tc.tile_pool(name="ot", bufs=8) as ot_pool:
        with (
            tc.tile_pool(name="wkv", bufs=1) as wkv_pool,
            tc.tile_pool(name="stgc", bufs=1) as stgc,
            tc.tile_pool(name="kt", bufs=1) as kt_pool,
            tc.tile_pool(name="v", bufs=1) as v_pool,
            tc.tile_pool(name="e", bufs=2) as e_pool,
            tc.tile_pool(name="rc", bufs=2) as rc_pool,
        ):
            for g in range(2):
                # V for the group's 4 heads, augmented with a ones column:
                # vaug[:, st*260 + hl*65 + (0..63)] = V[st block, head hl]
                # vaug[:, st*260 + hl*65 + 64]      = 1.0
                wst = stgc.tile([P, NT_D * 256], F32, tag="wstg")
                nc.sync.dma_start(
                    out=wst[:, :].rearrange("p (dc m) -> p dc m", dc=NT_D),
                    in_=wv_r[:, :, g * 256:(g + 1) * 256],
                )
                wvg = wkv_pool.tile([P, NT_D * 256], DTM, tag="wv")
                nc.vector.tensor_copy(out=wvg[:], in_=wst[:])
                vaug = v_pool.tile([P, NT_S * 260], DTM, tag="vaug")
                nc.vector.tensor_copy(
                    out=vaug[:, :].rearrange("p (t h e) -> p t h e",
                                             t=NT_S, h=4)[:, :, :, 64:65],
                    in_=ones_f32[:, 0:1].broadcast_to([P, NT_S, 4, 1]),
                )
                for st in range(NT_S):
                    ps = pp.tile([P, 512], F32, tag="proj")
                    for dc in range(NT_D):
                        mm(ps[:, 0:256],
                           xT[:, dc * S + st * P:dc * S + (st + 1) * P],
                           wvg[:, dc * 256:(dc + 1) * 256],
                           start=(dc == 0), stop=False)
                    mm(ps[:, 0:256], ones_sb[0:1, 0:P],
                       bias_sb["bv"][0:1, g * 256:(g + 1) * 256],
                       start=False, stop=True)
                    dst = vaug[:, st * 260:(st + 1) * 260]
                    dst = dst.rearrange("p (h e) -> p h e", h=4)[:, :, 0:64]
                    nc.vector.tensor_copy(
                        out=dst,
                        in_=ps[:, 0:256].rearrange("p (h e) -> p h e", h=4),
                    )

                for pi in range(2):
                    pr = 2 * g + pi
                    wst = stgc.tile([P, NT_D * 256], F32, tag="wstg")
                    nc.sync.dma_start(
                        out=wst[:, 0:NT_D * P].rearrange("p (dc m) -> p dc m",
                                                         dc=NT_D),
                        in_=wk_r[:, :, pr * P:(pr + 1) * P],
                    )
                    wkp = wkv_pool.tile([P, NT_D * P], DTM, tag="wk")
                    nc.vector.tensor_copy(out=wkp[:], in_=wst[:, 0:NT_D * P])
                    kt = kt_pool.tile([P, S], DTM, tag="KT")
                    for sc in range(8):
                        ps = pp.tile([P, 512], F32, tag="proj")
                        for dc in range(NT_D):
                            mm(ps[:], wkp[:, dc * P:(dc + 1) * P],
                               xT[:, dc * S + sc * 512:dc * S + (sc + 1) * 512],
                               start=(dc == 0), stop=False)
                        mm(ps[:], bias_sb["bk"][0:1, pr * P:(pr + 1) * P],
                           ones_sb[0:1, :], start=False, stop=True)
                        nc.vector.tensor_copy(out=kt[:, sc * 512:(sc + 1) * 512],
                                              in_=ps[:])

                    ot0 = ot_pool.tile([64, QL], DTM, tag="OT")
                    ot1 = ot_pool.tile([64, QL], DTM, tag="OT")
                    OT += [ot0, ot1]
                    qt = QT[pr]
                    for qc in range(QC):
                        o0 = o_pool.tile([65, 512], F32, tag="O")
                        o1 = o_pool.tile([65, 512], F32, tag="O")
                        for sk in range(NT_S // 2):
                            sp0 = sp_pool.tile([P, 1024], F32, tag="sc")
                            sp1 = sp_pool.tile([P, 1024], F32, tag="sc")
                            for j in range(2):
                                ktile = sk * 2 + j
                                ksl = slice(ktile * P, (ktile + 1) * P)
                                qsl = slice(qc * 512, (qc + 1) * 512)
                                # head 0 on PE rows 0-63, head 1 on 64-127
                                mm(sp0[:, j * 512:(j + 1) * 512],
                                   kt[0:64, ksl], qt[0:64, qsl])
                                mm(sp1[:, j * 512:(j + 1) * 512],
                                   kt[64:128, ksl], qt[64:128, qsl])
                            ea0 = e_pool.tile([P, 1024], DTM, tag="ea")
                            ea1 = e_pool.tile([P, 1024], DTM, tag="ea")
                            nc.scalar.activation(ea0[:], sp0[:], EXP, scale=0.125)
                            nc.scalar.activation(ea1[:], sp1[:], EXP, scale=0.125)
                            for j in range(2):
                                ktile = sk * 2 + j
                                st_ = ktile * 260
                                esl = slice(j * 512, (j + 1) * 512)
                                fl = dict(start=(ktile == 0),
                                          stop=(ktile == NT_S - 1))
                                hl0, hl1 = 2 * pi, 2 * pi + 1
                                mm(o0[:], vaug[:, st_ + hl0 * 65:
                                               st_ + hl0 * 65 + 65],
                                   ea0[:, esl], **fl)
                                mm(o1[:], vaug[:, st_ + hl1 * 65:
                                               st_ + hl1 * 65 + 65],
                                   ea1[:, esl], **fl)
                        # normalize: O[0:64] * (1 / O[64]) broadcast down.
                        # This small path stays plain fp32.
                        for o_ps, ot in ((o0, ot0), (o1, ot1)):
                            rc = rc_pool.tile([65, 512], F32, tag="rc")
                            nc.vector.reciprocal(out=rc[64:65, :],
                                                 in_=o_ps[64:65, :])
                            bc = pp.tile([P, 512], F32, tag="proj")
                            mm(bc[0:64, :], ones64_sb[64:65, :], rc[64:65, :])
                            bc_sb = rc_pool.tile([64, 512], F32, tag="bc")
                            nc.vector.tensor_copy(out=bc_sb[:], in_=bc[0:64, :])
                            nc.vector.tensor_mul(
                                ot[:, qc * 512:(qc + 1) * 512],
                                o_ps[0:64, :], bc_sb[:],
                            )

        # ---- stage D: output projection Y = concat_h(O_h) @ Wo + bo ------
        with (
            tc.tile_pool(name="wo", bufs=8) as wo_pool,
            tc.tile_pool(name="y", bufs=2) as y_pool,
        ):
            wo_sb = []
            for h in range(H):
                wst = y_pool.tile([64, D], F32, tag="wostg")
                nc.sync.dma_start(out=wst[:], in_=wo[h * 64:(h + 1) * 64, :])
                woh = wo_pool.tile([64, D], DTM, tag="wo")
                nc.vector.tensor_copy(out=woh[:], in_=wst[:])
                wo_sb.append(woh)
            for qt_i in range(QL // P):
                ps = pp.tile([P, 512], F32, tag="proj")
                for h in range(H):
                    mm(ps[:], OT[h][:, qt_i * P:(qt_i + 1) * P], wo_sb[h][:],
                       start=(h == 0), stop=False)
                mm(ps[:], ones_sb[0:1, 0:P], bias_sb["bo"][0:1, :],
                   start=False, stop=True)
                ysb = y_pool.tile([P, D], F32, tag="y")
                nc.vector.tensor_copy(out=ysb[:], in_=ps[:])
                nc.sync.dma_start(out=out[qt_i * P:(qt_i + 1) * P, :], in_=ysb[:])


def build():
    nc = bacc.Bacc("TRN2", target_bir_lowering=False, debug=False,
                   num_devices=N_CORES)
    io = {}
    for nm, shape in (("xb", [S, D]), ("xq", [QL, D]), ("wq", [D, D]),
                      ("wk", [D, D]), ("wv", [D, D]), ("wo", [D, D]),
                      ("bq", [1, D]), ("bk", [1, D]), ("bv", [1, D]),
                      ("bo", [1, D]), ("ident", [P, P])):
        io[nm] = nc.dram_tensor(nm, shape, F32, kind="ExternalInput").ap()
    io["out"] = nc.dram_tensor("out", [QL, D], F32, kind="ExternalOutput").ap()
    with tile.TileContext(nc) as tc:
        with ExitStack() as ctx:
            _emit(ctx, tc, io)
    nc.compile()
    return nc


def make_in_maps(inputs):
    f = lambda a: np.ascontiguousarray(np.asarray(a, dtype=np.float32))
    x = f(inputs["x"])
    fixed = {
        "wq": f(inputs["Wq"]), "wk": f(inputs["Wk"]), "wv": f(inputs["Wv"]),
        "wo": f(inputs["Wo"]),
        "bq": f(inputs["bq"]).reshape(1, D), "bk": f(inputs["bk"]).reshape(1, D),
        "bv": f(inputs["bv"]).reshape(1, D), "bo": f(inputs["bo"]).reshape(1, D),
        "ident": np.eye(P, dtype=np.float32),
    }
    in_maps = []
    for c in range(N_CORES):
        b, qs = c // 4, (c % 4) * QL
        in_maps.append({"xb": x[b], "xq": x[b, qs:qs + QL], **fixed})
    return in_maps


_CACHE = {}
LAST_EXEC_NS = None


def run(inputs, trace=False):
    global LAST_EXEC_NS
    if "nc" not in _CACHE:
        _CACHE["nc"] = build()
    nc = _CACHE["nc"]
    kw = {}
    if trace:
        import sys, types
        if "antenv.axon_hooks" not in sys.modules:
            sys.path.insert(0, "/root/.axon_site")
            try:
                from trn_agent_boot.trn_boot import _ntff_profile_via_ctypes
                hook = _ntff_profile_via_ctypes("/opt/axon/libaxon_pjrt.so")
                mod = types.ModuleType("antenv.axon_hooks")
                mod.get_axon_ntff_profile_hook = lambda: hook
                mod.set_axon_ntff_profile_hook = lambda h: None
                sys.modules["antenv.axon_hooks"] = mod
            except Exception:
                pass
        kw = dict(trace=True, trace_cores=[0])
    res = run_bass_kernel_spmd(nc, make_in_maps(inputs),
                               core_ids=list(range(N_CORES)), **kw)
    if trace:
        LAST_EXEC_NS = res.exec_time_ns
    out = np.empty((B, S, D), np.float32)
    for c in range(N_CORES):
        b, qs = c // 4, (c % 4) * QL
        out[b, qs:qs + QL] = res.results[c]["out"]
    return out


def kernel(**inputs) -> np.ndarray:
    return run(inputs, trace=False)


# revision 6
# speedup vs baseline: 1.2591x; 1.2591x over previous
"""Multi-head self-attention Trainium2 Bass kernel (8-core SPMD).

Sharding: data-parallel over query rows. The flattened (B*S, D) = (8192, 512)
query space is split into 8 blocks of 1024 rows; core c handles batch c//4,
query rows (c%4)*1024 .. +1024. Each core recomputes K/V for its whole batch
(4-way duplicated) which avoids any cross-core communication; host-side
gather is a pure concatenation.

Layout strategy: activations live transposed in SBUF ([D, S], d on
partitions). Projections then need no weight transposes:
  K^T = Wk^T x^T   (lhsT = Wk chunk, rhs = x^T chunk)
  V   = x Wv       (lhsT = x^T chunk, rhs = Wv chunk)
Scores are computed transposed ([k, q], k on partitions) so softmax's
denominator comes from a ones-column appended to V (row 64 of the attention
output accumulator), and A^T is directly consumable by the A@V matmul.
exp() runs on the scalar engine with the 1/sqrt(dk) folded into its scale.
The normalized per-head outputs O^T are exactly the lhsT the output
projection wants, so no transposes are needed anywhere except on the input x.

Matmul operands are stored as float32r (the PE's full-rate fp32 streaming
format, ~13 mantissa bits): every producer is an engine op whose output
dtype is f32r, which satisfies the BIR verifier's rounding requirement.
DMA-loaded operands (weights/biases) pass through a converting copy.
"""

from contextlib import ExitStack

import numpy as np

import concourse.bass as bass
import concourse.tile as tile
from concourse import bacc, mybir
from concourse.bass_utils import run_bass_kernel_spmd

N_CORES = 8
B, S, D, H, DK = 2, 4096, 512, 8, 64
QL = B * S // N_CORES          # 1024 query rows per core
P = 128
NT_S = S // P                  # 32 sequence tiles
NT_D = D // P                  # 4 d-model chunks
QC = QL // 512                 # 2 query chunks of 512
F32 = mybir.dt.float32
F32R = mybir.dt.float32r
EXP = mybir.ActivationFunctionType.Exp

F16 = mybir.dt.float16
# dtype for matmul operands: "f32r" (full-rate fp32, ~13 mantissa bits but
# pinned at the 1.2 GHz throttled clock), "f16" (10 mantissa bits, true
# 2.4 GHz MAC path + fast weight load), or "f32" (exact, 4 cycles/row).
MM_DTYPE = "f16"
DTM = {"f32r": F32R, "f16": F16, "f32": F32}[MM_DTYPE]


def _emit(ctx: ExitStack, tc: tile.TileContext, io: dict):
    nc = tc.nc
    xb, xq = io["xb"], io["xq"]
    wq, wk, wv, wo = io["wq"], io["wk"], io["wv"], io["wo"]
    bq, bk, bv, bo = io["bq"], io["bk"], io["bv"], io["bo"]
    ident = io["ident"]
    out = io["out"]

    mm = nc.tensor.matmul

    # ---- pools persistent across the whole kernel ------------------------
    consts = ctx.enter_context(tc.tile_pool(name="consts", bufs=1))
    xt_pool = ctx.enter_context(tc.tile_pool(name="xt", bufs=1))
    qt_pool = ctx.enter_context(tc.tile_pool(name="qt", bufs=4))
    # PSUM: proj/transpose/bias-bcast/outproj pool (1 bank x2), scores
    # (2 banks x2), attention accumulators (1 bank x2) -> exactly 8 banks.
    pp = ctx.enter_context(tc.tile_pool(name="pp", bufs=2, space="PSUM"))
    sp_pool = ctx.enter_context(tc.tile_pool(name="sp", bufs=2, space="PSUM"))
    o_pool = ctx.enter_context(tc.tile_pool(name="o", bufs=2, space="PSUM"))

    # ---- constants --------------------------------------------------------
    ident_sb = consts.tile([P, P], F32, tag="ident")
    nc.sync.dma_start(out=ident_sb[:], in_=ident[:])
    ones_f32 = consts.tile([P, 1], F32, tag="ones_f32")
    nc.vector.memset(ones_f32[:], 1.0)
    ones_sb = consts.tile([1, 512], DTM, tag="ones")
    nc.vector.tensor_copy(out=ones_sb[:], in_=ones_f32[0:1, 0:1].broadcast_to([1, 512]))
    # a f32 ones row living on partition 64 (denominator broadcast lhsT)
    ones64_sb = consts.tile([65, 64], F32, tag="ones64")
    nc.vector.memset(ones64_sb[64:65, :], 1.0)
    bias_sb = {}
    with tc.tile_pool(name="stg0", bufs=2) as stg0:
        for nm, ap in (("bq", bq), ("bk", bk), ("bv", bv), ("bo", bo)):
            st = stg0.tile([1, D], F32, tag="bstg")
            nc.sync.dma_start(out=st[:], in_=ap[:])
            t = consts.tile([1, D], DTM, tag=nm)
            nc.vector.tensor_copy(out=t[:], in_=st[:])
            bias_sb[nm] = t

    xT = xt_pool.tile([P, NT_D * S], DTM, tag="xT")
    wq_r = wq.rearrange("(dc p) m -> p dc m", p=P)
    wk_r = wk.rearrange("(dc p) m -> p dc m", p=P)
    wv_r = wv.rearrange("(dc p) m -> p dc m", p=P)
    QT = []

    with tc.tile_pool(name="xq", bufs=1) as xq_pool:
        xqT = xq_pool.tile([P, NT_D * QL], DTM, tag="xqT")

        # ---- stage A: x^T and xq^T via PE transposes ---------------------
        with tc.tile_pool(name="xn", bufs=3) as xn_pool:
            for src_ap, ntile, dst in ((xb, NT_S, xT), (xq, QL // P, xqT)):
                for st in range(ntile):
                    xn = xn_pool.tile([P, D], F32, tag="xn")
                    nc.sync.dma_start(out=xn[:], in_=src_ap[st * P:(st + 1) * P, :])
                    tp = pp.tile([P, 512], F32, tag="proj")
                    for dc in range(NT_D):
                        nc.tensor.transpose(
                            tp[:, dc * P:(dc + 1) * P],
                            xn[:, dc * P:(dc + 1) * P],
                            ident_sb[:],
                        )
                    dst_ap = dst[:, :].rearrange("p (dc s) -> p dc s", dc=NT_D)
                    nc.vector.tensor_copy(
                        out=dst_ap[:, :, st * P:(st + 1) * P],
                        in_=tp[:, :].rearrange("p (dc j) -> p dc j", dc=NT_D),
                    )

        # ---- stage B: Q^T for all 4 head pairs ---------------------------
        with (
            tc.tile_pool(name="wqp", bufs=1) as wq_pool,
            tc.tile_pool(name="stgb", bufs=2) as stgb,
        ):
            for pr in range(4):
                wst = stgb.tile([P, NT_D * P], F32, tag="wstg")
                nc.sync.dma_start(
                    out=wst[:, :].rearrange("p (dc m) -> p dc m", dc=NT_D),
                    in_=wq_r[:, :, pr * P:(pr + 1) * P],
                )
                wqp = wq_pool.tile([P, NT_D * P], DTM, tag="wq")
                nc.vector.tensor_copy(out=wqp[:], in_=wst[:])
                qt = qt_pool.tile([P, QL], DTM, tag="QT")
                for qc in range(QC):
                    ps = pp.tile([P, 512], F32, tag="proj")
                    for dc in range(NT_D):
                        mm(ps[:], wqp[:, dc * P:(dc + 1) * P],
                           xqT[:, dc * QL + qc * 512:dc * QL + (qc + 1) * 512],
                           start=(dc == 0), stop=False)
                    mm(ps[:], bias_sb["bq"][0:1, pr * P:(pr + 1) * P],
                       ones_sb[0:1, :], start=False, stop=True)
                    nc.vector.tensor_copy(out=qt[:, qc * 512:(qc + 1) * 512],
                                          in_=ps[:])
                QT.append(qt)

    # ---- stage C: per 4-head group: V, then per pair K^T + attention -----
    OT = []  # per-head [64, QL] normalized attention outputs (transposed)
    with tc.tile_pool(name="ot", bufs=8) as ot_pool:
        with (
            tc.tile_pool(name="wkv", bufs=1) as wkv_pool,
            tc.tile_pool(name="stgc", bufs=1) as stgc,
            tc.tile_pool(name="kt", bufs=1) as kt_pool,
            tc.tile_pool(name="v", bufs=1) as v_pool,
            tc.tile_pool(name="e", bufs=2) as e_pool,
            tc.tile_pool(name="rc", bufs=2) as rc_pool,
        ):
            for g in range(2):
                # V for the group's 4 heads, augmented with a ones column:
                # vaug[:, st*260 + hl*65 + (0..63)] = V[st block, head hl]
                # vaug[:, st*260 + hl*65 + 64]      = 1.0
                wst = stgc.tile([P, NT_D * 256], F32, tag="wstg")
                nc.sync.dma_start(
                    out=wst[:, :].rearrange("p (dc m) -> p dc m", dc=NT_D),
                    in_=wv_r[:, :, g * 256:(g + 1) * 256],
                )
                wvg = wkv_pool.tile([P, NT_D * 256], DTM, tag="wv")
                nc.vector.tensor_copy(out=wvg[:], in_=wst[:])
                vaug = v_pool.tile([P, NT_S * 260], DTM, tag="vaug")
                nc.vector.tensor_copy(
                    out=vaug[:, :].rearrange("p (t h e) -> p t h e",
                                             t=NT_S, h=4)[:, :, :, 64:65],
                    in_=ones_f32[:, 0:1].broadcast_to([P, NT_S, 4, 1]),
                )
                for st in range(NT_S):
                    ps = pp.tile([P, 512], F32, tag="proj")
                    for dc in range(NT_D):
                        mm(ps[:, 0:256],
                           xT[:, dc * S + st * P:dc * S + (st + 1) * P],
                           wvg[:, dc * 256:(dc + 1) * 256],
                           start=(dc == 0), stop=False)
                    mm(ps[:, 0:256], ones_sb[0:1, 0:P],
                       bias_sb["bv"][0:1, g * 256:(g + 1) * 256],
                       start=False, stop=True)
                    dst = vaug[:, st * 260:(st + 1) * 260]
                    dst = dst.rearrange("p (h e) -> p h e", h=4)[:, :, 0:64]
                    nc.vector.tensor_copy(
                        out=dst,
                        in_=ps[:, 0:256].rearrange("p (h e) -> p h e", h=4),
                    )

                for pi in range(2):
                    pr = 2 * g + pi
                    wst = stgc.tile([P, NT_D * 256], F32, tag="wstg")
                    nc.sync.dma_start(
                        out=wst[:, 0:NT_D * P].rearrange("p (dc m) -> p dc m",
                                                         dc=NT_D),
                        in_=wk_r[:, :, pr * P:(pr + 1) * P],
                    )
                    wkp = wkv_pool.tile([P, NT_D * P], DTM, tag="wk")
                    nc.vector.tensor_copy(out=wkp[:], in_=wst[:, 0:NT_D * P])
                    kt = kt_pool.tile([P, S], DTM, tag="KT")
                    for sc in range(8):
                        ps = pp.tile([P, 512], F32, tag="proj")
                        for dc in range(NT_D):
                            mm(ps[:], wkp[:, dc * P:(dc + 1) * P],
                               xT[:, dc * S + sc * 512:dc * S + (sc + 1) * 512],
                               start=(dc == 0), stop=False)
                        mm(ps[:], bias_sb["bk"][0:1, pr * P:(pr + 1) * P],
                           ones_sb[0:1, :], start=False, stop=True)
                        nc.vector.tensor_copy(out=kt[:, sc * 512:(sc + 1) * 512],
                                              in_=ps[:])

                    ot0 = ot_pool.tile([64, QL], DTM, tag="OT")
                    ot1 = ot_pool.tile([64, QL], DTM, tag="OT")
                    OT += [ot0, ot1]
                    qt = QT[pr]
                    for qc in range(QC):
                        o0 = o_pool.tile([65, 512], F32, tag="O")
                        o1 = o_pool.tile([65, 512], F32, tag="O")
                        for sk in range(NT_S // 2):
                            sp0 = sp_pool.tile([P, 1024], F32, tag="sc")
                            sp1 = sp_pool.tile([P, 1024], F32, tag="sc")
                            for j in range(2):
                                ktile = sk * 2 + j
                                ksl = slice(ktile * P, (ktile + 1) * P)
                                qsl = slice(qc * 512, (qc + 1) * 512)
                                # head 0 on PE rows 0-63, head 1 on 64-127
                                mm(sp0[:, j * 512:(j + 1) * 512],
                                   kt[0:64, ksl], qt[0:64, qsl])
                                mm(sp1[:, j * 512:(j + 1) * 512],
                                   kt[64:128, ksl], qt[64:128, qsl])
                            ea0 = e_pool.tile([P, 1024], DTM, tag="ea")
                            ea1 = e_pool.tile([P, 1024], DTM, tag="ea")
                            nc.scalar.activation(ea0[:], sp0[:], EXP, scale=0.125)
                            nc.scalar.activation(ea1[:], sp1[:], EXP, scale=0.125)
                            for j in range(2):
                                ktile = sk * 2 + j
                                st_ = ktile * 260
                                esl = slice(j * 512, (j + 1) * 512)
                                fl = dict(start=(ktile == 0),
                                          stop=(ktile == NT_S - 1))
                                hl0, hl1 = 2 * pi, 2 * pi + 1
                                mm(o0[:], vaug[:, st_ + hl0 * 65:
                                               st_ + hl0 * 65 + 65],
                                   ea0[:, esl], **fl)
                                mm(o1[:], vaug[:, st_ + hl1 * 65:
                                               st_ + hl1 * 65 + 65],
                                   ea1[:, esl], **fl)
                        # normalize: O[0:64] * (1 / O[64]) broadcast down.
                        # This small path stays plain fp32.
                        for o_ps, ot in ((o0, ot0), (o1, ot1)):
                            rc = rc_pool.tile([65, 512], F32, tag="rc")
                            nc.vector.reciprocal(out=rc[64:65, :],
                                                 in_=o_ps[64:65, :])
                            bc = pp.tile([P, 512], F32, tag="proj")
                            mm(bc[0:64, :], ones64_sb[64:65, :], rc[64:65, :])
                            bc_sb = rc_pool.tile([64, 512], F32, tag="bc")
                            nc.vector.tensor_copy(out=bc_sb[:], in_=bc[0:64, :])
                            nc.vector.tensor_mul(
                                ot[:, qc * 512:(qc + 1) * 512],
                                o_ps[0:64, :], bc_sb[:],
                            )

        # ---- stage D: output projection Y = concat_h(O_h) @ Wo + bo ------
        with (
            tc.tile_pool(name="wo", bufs=8) as wo_pool,
            tc.tile_pool(name="y", bufs=2) as y_pool,
        ):
            wo_sb = []
            for h in range(H):
                wst = y_pool.tile([64, D], F32, tag="wostg")
                nc.sync.dma_start(out=wst[:], in_=wo[h * 64:(h + 1) * 64, :])
                woh = wo_pool.tile([64, D], DTM, tag="wo")
                nc.vector.tensor_copy(out=woh[:], in_=wst[:])
                wo_sb.append(woh)
            for qt_i in range(QL // P):
                ps = pp.tile([P, 512], F32, tag="proj")
                for h in range(H):
                    mm(ps[:], OT[h][:, qt_i * P:(qt_i + 1) * P], wo_sb[h][:],
                       start=(h == 0), stop=False)
                mm(ps[:], ones_sb[0:1, 0:P], bias_sb["bo"][0:1, :],
                   start=False, stop=True)
                ysb = y_pool.tile([P, D], F32, tag="y")
                nc.vector.tensor_copy(out=ysb[:], in_=ps[:])
                nc.sync.dma_start(out=out[qt_i * P:(qt_i + 1) * P, :], in_=ysb[:])


def build():
    nc = bacc.Bacc("TRN2", target_bir_lowering=False, debug=False,
                   num_devices=N_CORES)
    io = {}
    for nm, shape in (("xb", [S, D]), ("xq", [QL, D]), ("wq", [D, D]),
                      ("wk", [D, D]), ("wv", [D, D]), ("wo", [D, D]),
                      ("bq", [1, D]), ("bk", [1, D]), ("bv", [1, D]),
                      ("bo", [1, D]), ("ident", [P, P])):
        io[nm] = nc.dram_tensor(nm, shape, F32, kind="ExternalInput").ap()
    io["out"] = nc.dram_tensor("out", [QL, D], F32, kind="ExternalOutput").ap()
    with tile.TileContext(nc) as tc:
        with ExitStack() as ctx:
            _emit(ctx, tc, io)
    nc.compile()
    return nc


def make_in_maps(inputs):
    f = lambda a: np.ascontiguousarray(np.asarray(a, dtype=np.float32))
    x = f(inputs["x"])
    fixed = {
        "wq": f(inputs["Wq"]), "wk": f(inputs["Wk"]), "wv": f(inputs["Wv"]),
        "wo": f(inputs["Wo"]),
        "bq": f(inputs["bq"]).reshape(1, D), "bk": f(inputs["bk"]).reshape(1, D),
        "bv": f(inputs["bv"]).reshape(1, D), "bo": f(inputs["bo"]).reshape(1, D),
        "ident": np.eye(P, dtype=np.float32),
    }
    in_maps = []
    for c in range(N_CORES):
        b, qs = c // 4, (c % 4) * QL
        in_maps.append({"xb": x[b], "xq": x[b, qs:qs + QL], **fixed})
    return in_maps


_CACHE = {}
LAST_EXEC_NS = None


def run(inputs, trace=False):
    global LAST_EXEC_NS
    if "nc" not in _CACHE:
        _CACHE["nc"] = build()
    nc = _CACHE["nc"]
    kw = {}
    if trace:
        import sys, types
        if "antenv.axon_hooks" not in sys.modules:
            sys.path.insert(0, "/root/.axon_site")
            try:
                from trn_agent_boot.trn_boot import _ntff_profile_via_ctypes
                hook = _ntff_profile_via_ctypes("/opt/axon/libaxon_pjrt.so")
                mod = types.ModuleType("antenv.axon_hooks")
                mod.get_axon_ntff_profile_hook = lambda: hook
                mod.set_axon_ntff_profile_hook = lambda h: None
                sys.modules["antenv.axon_hooks"] = mod
            except Exception:
                pass
        kw = dict(trace=True, trace_cores=[0])
    res = run_bass_kernel_spmd(nc, make_in_maps(inputs),
                               core_ids=list(range(N_CORES)), **kw)
    if trace:
        LAST_EXEC_NS = res.exec_time_ns
    out = np.empty((B, S, D), np.float32)
    for c in range(N_CORES):
        b, qs = c // 4, (c % 4) * QL
        out[b, qs:qs + QL] = res.results[c]["out"]
    return out


def kernel(**inputs) -> np.ndarray:
    return run(inputs, trace=False)


# revision 8
# speedup vs baseline: 1.3125x; 1.0424x over previous
"""Multi-head self-attention Trainium2 Bass kernel (8-core SPMD).

Sharding: data-parallel over query rows. The flattened (B*S, D) = (8192, 512)
query space is split into 8 blocks of 1024 rows; core c handles batch c//4,
query rows (c%4)*1024 .. +1024. Each core recomputes K/V for its whole batch
(4-way duplicated) which avoids any cross-core communication; host-side
gather is a pure concatenation.

Layout strategy: activations live transposed in SBUF ([D, S], d on
partitions). Projections then need no weight transposes:
  K^T = Wk^T x^T   (lhsT = Wk chunk, rhs = x^T chunk)
  V   = x Wv       (lhsT = x^T chunk, rhs = Wv chunk)
Scores are computed transposed ([k, q], k on partitions) so softmax's
denominator comes from a ones-column appended to V (row 64 of the attention
output accumulator), and A^T is directly consumable by the A@V matmul.
exp() runs on the scalar engine with the 1/sqrt(dk) folded into its scale.
The normalized per-head outputs O^T are exactly the lhsT the output
projection wants, so no transposes are needed anywhere except on the input x.

Matmul operands are stored as fp16 (10-bit mantissa; measured end-to-end
absmax relative error ~4e-4): unlike f32r this uses the true MAC path, so
the PE clock-gate (HAM) warms to 2.4 GHz and fast weight load applies.
All accumulation is fp32 in PSUM; softmax denominators/reciprocals are fp32.

PSUM budget (8 banks): one shared pool of [128,512] tiles (bufs=6) serves
transposes, projections, scores, the denominator broadcast and the output
projection; a 2-buffer pool holds the per-head attention accumulators.
"""

from contextlib import ExitStack

import numpy as np

import concourse.bass as bass
import concourse.tile as tile
from concourse import bacc, mybir
from concourse.bass_utils import run_bass_kernel_spmd

N_CORES = 8
B, S, D, H, DK = 2, 4096, 512, 8, 64
QL = B * S // N_CORES          # 1024 query rows per core
P = 128
NT_S = S // P                  # 32 sequence tiles
NT_D = D // P                  # 4 d-model chunks
QC = QL // 512                 # 2 query chunks of 512
F32 = mybir.dt.float32
F32R = mybir.dt.float32r
F16 = mybir.dt.float16
EXP = mybir.ActivationFunctionType.Exp

# "f16" (10 mantissa bits, 2.4 GHz MAC path + FWL), "f32r" (13 bits but
# pinned at the 1.2 GHz throttled clock), "f32" (exact, 4 cycles/row).
MM_DTYPE = "f16"
DTM = {"f32r": F32R, "f16": F16, "f32": F32}[MM_DTYPE]


def _emit(ctx: ExitStack, tc: tile.TileContext, io: dict):
    nc = tc.nc
    xb, xq = io["xb"], io["xq"]
    wq, wk, wv, wo = io["wq"], io["wk"], io["wv"], io["wo"]
    bq, bk, bv, bo = io["bq"], io["bk"], io["bv"], io["bo"]
    ident = io["ident"]
    out = io["out"]

    mm = nc.tensor.matmul

    # ---- pools persistent across the whole kernel ------------------------
    consts = ctx.enter_context(tc.tile_pool(name="consts", bufs=1))
    xt_pool = ctx.enter_context(tc.tile_pool(name="xt", bufs=1))
    qt_pool = ctx.enter_context(tc.tile_pool(name="qt", bufs=4))
    # PSUM: shared [128,512] pool (6 banks) + attention accumulators (2).
    ps_pool = ctx.enter_context(tc.tile_pool(name="ps", bufs=6, space="PSUM"))
    o_pool = ctx.enter_context(tc.tile_pool(name="o", bufs=2, space="PSUM"))

    def psum512():
        return ps_pool.tile([P, 512], F32, tag="ps", name="ps")

    # ---- constants --------------------------------------------------------
    ident_sb = consts.tile([P, P], F32, tag="ident")
    nc.sync.dma_start(out=ident_sb[:], in_=ident[:])
    ones_f32 = consts.tile([P, 1], F32, tag="ones_f32")
    nc.vector.memset(ones_f32[:], 1.0)
    ones_sb = consts.tile([1, 512], DTM, tag="ones")
    nc.vector.tensor_copy(out=ones_sb[:], in_=ones_f32[0:1, 0:1].broadcast_to([1, 512]))
    # a f32 ones row living on partition 64 (denominator broadcast lhsT)
    ones64_sb = consts.tile([65, 64], F32, tag="ones64")
    nc.vector.memset(ones64_sb[64:65, :], 1.0)
    bias_sb = {}
    with tc.tile_pool(name="stg0", bufs=2) as stg0:
        for nm, ap in (("bq", bq), ("bk", bk), ("bv", bv), ("bo", bo)):
            st = stg0.tile([1, D], F32, tag="bstg")
            nc.sync.dma_start(out=st[:], in_=ap[:])
            t = consts.tile([1, D], DTM, tag=nm)
            nc.vector.tensor_copy(out=t[:], in_=st[:])
            bias_sb[nm] = t

    xT = xt_pool.tile([P, NT_D * S], DTM, tag="xT")
    wq_r = wq.rearrange("(dc p) m -> p dc m", p=P)
    wk_r = wk.rearrange("(dc p) m -> p dc m", p=P)
    wv_r = wv.rearrange("(dc p) m -> p dc m", p=P)
    QT = []

    with tc.tile_pool(name="xq", bufs=1) as xq_pool:
        xqT = xq_pool.tile([P, NT_D * QL], DTM, tag="xqT")

        # ---- stage A: x^T and xq^T via PE transposes ---------------------
        with tc.tile_pool(name="xn", bufs=3) as xn_pool:
            for src_ap, ntile, dst in ((xb, NT_S, xT), (xq, QL // P, xqT)):
                for st in range(ntile):
                    xn = xn_pool.tile([P, D], F32, tag="xn")
                    nc.sync.dma_start(out=xn[:], in_=src_ap[st * P:(st + 1) * P, :])
                    tp = psum512()
                    for dc in range(NT_D):
                        nc.tensor.transpose(
                            tp[:, dc * P:(dc + 1) * P],
                            xn[:, dc * P:(dc + 1) * P],
                            ident_sb[:],
                        )
                    dst_ap = dst[:, :].rearrange("p (dc s) -> p dc s", dc=NT_D)
                    nc.vector.tensor_copy(
                        out=dst_ap[:, :, st * P:(st + 1) * P],
                        in_=tp[:, :].rearrange("p (dc j) -> p dc j", dc=NT_D),
                    )

        # ---- stage B: Q^T for all 4 head pairs ---------------------------
        with (
            tc.tile_pool(name="wqp", bufs=1) as wq_pool,
            tc.tile_pool(name="stgb", bufs=2) as stgb,
        ):
            for pr in range(4):
                wst = stgb.tile([P, NT_D * P], F32, tag="wstg")
                nc.sync.dma_start(
                    out=wst[:, :].rearrange("p (dc m) -> p dc m", dc=NT_D),
                    in_=wq_r[:, :, pr * P:(pr + 1) * P],
                )
                wqp = wq_pool.tile([P, NT_D * P], DTM, tag="wq")
                nc.vector.tensor_copy(out=wqp[:], in_=wst[:])
                qt = qt_pool.tile([P, QL], DTM, tag="QT")
                for qc in range(QC):
                    ps = psum512()
                    for dc in range(NT_D):
                        mm(ps[:], wqp[:, dc * P:(dc + 1) * P],
                           xqT[:, dc * QL + qc * 512:dc * QL + (qc + 1) * 512],
                           start=(dc == 0), stop=False)
                    mm(ps[:], bias_sb["bq"][0:1, pr * P:(pr + 1) * P],
                       ones_sb[0:1, :], start=False, stop=True)
                    nc.vector.tensor_copy(out=qt[:, qc * 512:(qc + 1) * 512],
                                          in_=ps[:])
                QT.append(qt)

    # ---- stage C: per 4-head group: V, then per pair K^T + attention -----
    OT = []  # per-head [64, QL] normalized attention outputs (transposed)
    with tc.tile_pool(name="ot", bufs=8) as ot_pool:
        with (
            tc.tile_pool(name="wkv", bufs=1) as wkv_pool,
            tc.tile_pool(name="stgc", bufs=1) as stgc,
            tc.tile_pool(name="kt", bufs=1) as kt_pool,
            tc.tile_pool(name="v", bufs=1) as v_pool,
            tc.tile_pool(name="e", bufs=6) as e_pool,
            tc.tile_pool(name="rc", bufs=4) as rc_pool,
        ):
            for g in range(2):
                # V for the group's 4 heads, augmented with a ones column:
                # vaug[:, st*260 + hl*65 + (0..63)] = V[st block, head hl]
                # vaug[:, st*260 + hl*65 + 64]      = 1.0
                wst = stgc.tile([P, NT_D * 256], F32, tag="wstg")
                nc.sync.dma_start(
                    out=wst[:, :].rearrange("p (dc m) -> p dc m", dc=NT_D),
                    in_=wv_r[:, :, g * 256:(g + 1) * 256],
                )
                wvg = wkv_pool.tile([P, NT_D * 256], DTM, tag="wv")
                nc.vector.tensor_copy(out=wvg[:], in_=wst[:])
                vaug = v_pool.tile([P, NT_S * 260], DTM, tag="vaug")
                nc.vector.tensor_copy(
                    out=vaug[:, :].rearrange("p (t h e) -> p t h e",
                                             t=NT_S, h=4)[:, :, :, 64:65],
                    in_=ones_f32[:, 0:1].broadcast_to([P, NT_S, 4, 1]),
                )
                for st in range(NT_S):
                    ps = psum512()
                    for dc in range(NT_D):
                        mm(ps[:, 0:256],
                           xT[:, dc * S + st * P:dc * S + (st + 1) * P],
                           wvg[:, dc * 256:(dc + 1) * 256],
                           start=(dc == 0), stop=False)
                    mm(ps[:, 0:256], ones_sb[0:1, 0:P],
                       bias_sb["bv"][0:1, g * 256:(g + 1) * 256],
                       start=False, stop=True)
                    dst = vaug[:, st * 260:(st + 1) * 260]
                    dst = dst.rearrange("p (h e) -> p h e", h=4)[:, :, 0:64]
                    nc.vector.tensor_copy(
                        out=dst,
                        in_=ps[:, 0:256].rearrange("p (h e) -> p h e", h=4),
                    )

                for pi in range(2):
                    pr = 2 * g + pi
                    wst = stgc.tile([P, NT_D * 256], F32, tag="wstg")
                    nc.sync.dma_start(
                        out=wst[:, 0:NT_D * P].rearrange("p (dc m) -> p dc m",
                                                         dc=NT_D),
                        in_=wk_r[:, :, pr * P:(pr + 1) * P],
                    )
                    wkp = wkv_pool.tile([P, NT_D * P], DTM, tag="wk")
                    nc.vector.tensor_copy(out=wkp[:], in_=wst[:, 0:NT_D * P])
                    kt = kt_pool.tile([P, S], DTM, tag="KT")
                    for sc in range(8):
                        ps = psum512()
                        for dc in range(NT_D):
                            mm(ps[:], wkp[:, dc * P:(dc + 1) * P],
                               xT[:, dc * S + sc * 512:dc * S + (sc + 1) * 512],
                               start=(dc == 0), stop=False)
                        mm(ps[:], bias_sb["bk"][0:1, pr * P:(pr + 1) * P],
                           ones_sb[0:1, :], start=False, stop=True)
                        nc.vector.tensor_copy(out=kt[:, sc * 512:(sc + 1) * 512],
                                              in_=ps[:])

                    ot0 = ot_pool.tile([64, QL], DTM, tag="OT")
                    ot1 = ot_pool.tile([64, QL], DTM, tag="OT")
                    OT += [ot0, ot1]
                    qt = QT[pr]
                    hl0, hl1 = 2 * pi, 2 * pi + 1
                    for qc in range(QC):
                        qsl = slice(qc * 512, (qc + 1) * 512)
                        o0 = o_pool.tile([65, 512], F32, tag="O")
                        o1 = o_pool.tile([65, 512], F32, tag="O")
                        for ktile in range(NT_S):
                            ksl = slice(ktile * P, (ktile + 1) * P)
                            fl = dict(start=(ktile == 0),
                                      stop=(ktile == NT_S - 1))
                            st_ = ktile * 260
                            # heads ride PE row strips 0-63 / 64-127
                            sp0 = psum512()
                            sp1 = psum512()
                            mm(sp0[:], kt[0:64, ksl], qt[0:64, qsl])
                            mm(sp1[:], kt[64:128, ksl], qt[64:128, qsl])
                            ea0 = e_pool.tile([P, 512], DTM, tag="ea")
                            ea1 = e_pool.tile([P, 512], DTM, tag="ea")
                            nc.scalar.activation(ea0[:], sp0[:], EXP, scale=0.125)
                            nc.scalar.activation(ea1[:], sp1[:], EXP, scale=0.125)
                            mm(o0[:], vaug[:, st_ + hl0 * 65:st_ + hl0 * 65 + 65],
                               ea0[:], **fl)
                            mm(o1[:], vaug[:, st_ + hl1 * 65:st_ + hl1 * 65 + 65],
                               ea1[:], **fl)
                        # normalize: O[0:64] * (1 / O[64]) broadcast down.
                        # Copy O out of PSUM immediately (frees the bank),
                        # then run the denominator chain out of SBUF.
                        for o_ps, ot in ((o0, ot0), (o1, ot1)):
                            osb = rc_pool.tile([65, 512], F32, tag="osb")
                            nc.vector.tensor_copy(out=osb[:], in_=o_ps[:])
                            bc = psum512()
                            mm(bc[0:64, :], ones64_sb[64:65, :], osb[64:65, :])
                            rbc = rc_pool.tile([64, 512], F32, tag="rbc")
                            nc.vector.reciprocal(out=rbc[:], in_=bc[0:64, :])
                            nc.vector.tensor_mul(ot[:, qsl], osb[0:64, :], rbc[:])

        # ---- stage D: output projection Y = concat_h(O_h) @ Wo + bo ------
        with (
            tc.tile_pool(name="wo", bufs=8) as wo_pool,
            tc.tile_pool(name="y", bufs=2) as y_pool,
        ):
            wo_sb = []
            for h in range(H):
                wst = y_pool.tile([64, D], F32, tag="wostg")
                nc.sync.dma_start(out=wst[:], in_=wo[h * 64:(h + 1) * 64, :])
                woh = wo_pool.tile([64, D], DTM, tag="wo")
                nc.vector.tensor_copy(out=woh[:], in_=wst[:])
                wo_sb.append(woh)
            for qt_i in range(QL // P):
                ps = psum512()
                for h in range(H):
                    mm(ps[:], OT[h][:, qt_i * P:(qt_i + 1) * P], wo_sb[h][:],
                       start=(h == 0), stop=False)
                mm(ps[:], ones_sb[0:1, 0:P], bias_sb["bo"][0:1, :],
                   start=False, stop=True)
                ysb = y_pool.tile([P, D], F32, tag="y")
                nc.vector.tensor_copy(out=ysb[:], in_=ps[:])
                nc.sync.dma_start(out=out[qt_i * P:(qt_i + 1) * P, :], in_=ysb[:])


def build():
    nc = bacc.Bacc("TRN2", target_bir_lowering=False, debug=False,
                   num_devices=N_CORES)
    io = {}
    for nm, shape in (("xb", [S, D]), ("xq", [QL, D]), ("wq", [D, D]),
                      ("wk", [D, D]), ("wv", [D, D]), ("wo", [D, D]),
                      ("bq", [1, D]), ("bk", [1, D]), ("bv", [1, D]),
                      ("bo", [1, D]), ("ident", [P, P])):
        io[nm] = nc.dram_tensor(nm, shape, F32, kind="ExternalInput").ap()
    io["out"] = nc.dram_tensor("out", [QL, D], F32, kind="ExternalOutput").ap()
    with tile.TileContext(nc) as tc:
        with ExitStack() as ctx:
            _emit(ctx, tc, io)
    nc.compile()
    return nc


def make_in_maps(inputs):
    f = lambda a: np.ascontiguousarray(np.asarray(a, dtype=np.float32))
    x = f(inputs["x"])
    fixed = {
        "wq": f(inputs["Wq"]), "wk": f(inputs["Wk"]), "wv": f(inputs["Wv"]),
        "wo": f(inputs["Wo"]),
        "bq": f(inputs["bq"]).reshape(1, D), "bk": f(inputs["bk"]).reshape(1, D),
        "bv": f(inputs["bv"]).reshape(1, D), "bo": f(inputs["bo"]).reshape(1, D),
        "ident": np.eye(P, dtype=np.float32),
    }
    in_maps = []
    for c in range(N_CORES):
        b, qs = c // 4, (c % 4) * QL
        in_maps.append({"xb": x[b], "xq": x[b, qs:qs + QL], **fixed})
    return in_maps


_CACHE = {}
LAST_EXEC_NS = None


def run(inputs, trace=False):
    global LAST_EXEC_NS
    if "nc" not in _CACHE:
        _CACHE["nc"] = build()
    nc = _CACHE["nc"]
    kw = {}
    if trace:
        import sys, types
        if "antenv.axon_hooks" not in sys.modules:
            sys.path.insert(0, "/root/.axon_site")
            try:
                from trn_agent_boot.trn_boot import _ntff_profile_via_ctypes
                hook = _ntff_profile_via_ctypes("/opt/axon/libaxon_pjrt.so")
                mod = types.ModuleType("antenv.axon_hooks")
                mod.get_axon_ntff_profile_hook = lambda: hook
                mod.set_axon_ntff_profile_hook = lambda h: None
                sys.modules["antenv.axon_hooks"] = mod
            except Exception:
                pass
        kw = dict(trace=True, trace_cores=[0])
    res = run_bass_kernel_spmd(nc, make_in_maps(inputs),
                               core_ids=list(range(N_CORES)), **kw)
    if trace:
        LAST_EXEC_NS = res.exec_time_ns
    out = np.empty((B, S, D), np.float32)
    for c in range(N_CORES):
        b, qs = c // 4, (c % 4) * QL
        out[b, qs:qs + QL] = res.results[c]["out"]
    return out


def kernel(**inputs) -> np.ndarray:
    return run(inputs, trace=False)
